# revision 26
# baseline (speedup 1.0000x reference)
"""GraphWaveNet layer on 8 Trainium2 NeuronCores.

Sharding: nodes partitioned across 8 cores (1250 each, padded to 1280).
v2 design (vs v0 baseline at ~432us):
  Phase A (node-chunk-major, t-pair packed): gated causal conv with both
      tanh activations running full 128-partition width by packing t and
      t+1 into the two partition halves; gcn linear via a block-diagonal
      [[W,0],[0,W]] moving operand so the (t,t+1) pair needs one
      stationary load per data chunk.  Produces the fp8 hw slab
      [128, 10 groups x 768] partition-major in SBUF.
  AllGather (chunked, 3 chunks of 4/4/2 node groups): each chunk's slab
      panel is DMA'd to DRAM and all-gathered as soon as phase A finishes
      that node chunk, overlapping the collective with the rest of
      phase A and with phase C's start.  Gathered panels are loaded into
      a persistent SBUF table [128, 80 groups x 768] (61.4KB/partition)
      with one contiguous descriptor per (chunk, core).
  Phase C (dense adjacency matmul, table resident in SBUF): 3 passes of
      4/4/2 dst windows (PSUM: 2 banks per window).  The normalized dense
      adjacency MD streams from DRAM pass-major with partition-contiguous
      4KB descriptors and deep prefetch, so the PE runs back-to-back fp8
      DoubleRow matmuls (no HBM re-streaming of the table).  Fused
      permute + residual + bias epilogue per window.
"""

import os
import numpy as np
import ml_dtypes

import concourse.bass as bass
import concourse.bacc as bacc
import concourse.mybir as mybir
import concourse.tile as tile
from concourse.bass_utils import run_bass_kernel_spmd

N, C, T, E = 10000, 64, 12, 160000
NCORES = 8
NL = N // NCORES            # 1250 real nodes per core
NLP = 1280                  # padded nodes per core
NG = NCORES * NLP           # 10240 padded global nodes
ROW = C * T                 # 768 elems per hw row, (t-major, c-minor)
COLS = NLP * T              # 15360 cols, (t-major, n-minor): col = t*1280 + n
WINS = NLP // 128           # 10 dst windows per core
NGRP = NLP // 128           # 10 node groups per core
NPAIR = NG // 256           # 40 src row-pair blocks
GG = NG // 128              # 80 global table groups

CHUNKS = ((0, 4), (4, 4), (8, 2))   # (first group, ngroups) per AG chunk
PASS_WINS = (4, 4, 2)               # dst windows per PSUM pass
PCH = 4                             # pairs per MD stream chunk

F32 = mybir.dt.float32
BF16 = mybir.dt.bfloat16
FP8 = mybir.dt.float8e4
NP_BF16 = ml_dtypes.bfloat16
NP_FP8 = ml_dtypes.float8_e4m3

MD_BYTES = sum(NPAIR * 2 * nw * 128 for nw in PASS_WINS)  # per partition

LAST_EXEC_NS = None
LAST_RESULTS = None

_prog_cache = {}


def _build_program():
    AFT = mybir.ActivationFunctionType
    DR = mybir.MatmulPerfMode.DoubleRow

    nc = bacc.Bacc(None, num_devices=NCORES, dynamic_dma_scratch_size=16384)
    XS = nc.dram_tensor("xs", [128, COLS], BF16, kind="ExternalInput")
    WF = nc.dram_tensor("wf", [2 * C, C], BF16, kind="ExternalInput")
    WG = nc.dram_tensor("wg", [2 * C, C], BF16, kind="ExternalInput")
    BIAF = nc.dram_tensor("biaf", [128, 1], F32, kind="ExternalInput")
    BIAG = nc.dram_tensor("biag", [128, 1], F32, kind="ExternalInput")
    GWBD = nc.dram_tensor("gwbd", [128, 128], BF16, kind="ExternalInput")
    MD = nc.dram_tensor("md", [128, MD_BYTES], FP8, kind="ExternalInput")
    XR = nc.dram_tensor("xr", [128, NGRP * ROW], BF16, kind="ExternalInput")
    OUT = nc.dram_tensor("out", [NLP, ROW], F32, kind="ExternalOutput")

    with tile.TileContext(nc) as tc:
        with (
            tc.tile_pool(name="dram", bufs=1, space="DRAM") as dram,
            tc.tile_pool(name="const", bufs=1) as cp,
            tc.tile_pool(name="fo", bufs=3) as fo,
        ):
            slab_cs = [dram.tile([128, ng * 768], FP8, name=f"slab_c{i}")
                       for i, (g0, ng) in enumerate(CHUNKS)]
            hw_cs = [dram.tile([NCORES * 128, ng * 768], FP8,
                               addr_space="Shared", name=f"hw_c{i}")
                     for i, (g0, ng) in enumerate(CHUNKS)]

            # Dummy 2KB AllGather issued first: pays the ~35us collective
            # bootstrap latency concurrently with phase A so the real slab
            # AllGathers start promptly when their inputs are ready.
            warm_in = dram.tile([128, 16], FP8, name="warm_in")
            warm_out = dram.tile([NCORES * 128, 16], FP8,
                                 addr_space="Shared", name="warm_out")
            wtile = cp.tile([128, 16], FP8)
            nc.vector.memset(wtile[:], 0)
            nc.sync.dma_start(warm_in[:], wtile[:])
            nc.gpsimd.collective_compute(
                "AllGather",
                mybir.AluOpType.bypass,
                replica_groups=[list(range(NCORES))],
                ins=[warm_in.opt()],
                outs=[warm_out.opt()],
            )

            # small weights first (they unblock the first conv ldweights),
            # then xs per node-chunk (strided over t) so the first chunk's
            # convs start after ~1/3 of the transfer, then the bulky xr.
            wf_sb = cp.tile([2 * C, C], BF16)
            wg_sb = cp.tile([2 * C, C], BF16)
            biaf_sb = cp.tile([128, 1], F32)
            biag_sb = cp.tile([128, 1], F32)
            gwbd_sb = cp.tile([128, 128], BF16)
            for t_, d_ in ((wf_sb, WF), (wg_sb, WG), (biaf_sb, BIAF),
                           (biag_sb, BIAG), (gwbd_sb, GWBD)):
                nc.sync.dma_start(t_[:], d_[:])

            xs_sb = cp.tile([128, COLS], BF16)
            xs3 = xs_sb[:].rearrange("p (t n) -> p t n", n=NLP)
            XS3 = XS[:].rearrange("p (t n) -> p t n", n=NLP)
            for g0, ng in CHUNKS:
                nc.sync.dma_start(
                    xs3[:, :, g0 * 128 : (g0 + ng) * 128],
                    XS3[:, :, g0 * 128 : (g0 + ng) * 128],
                )

            xr_sb = cp.tile([128, NGRP * ROW], BF16)
            nc.sync.dma_start(xr_sb[:], XR[:])

            slab_sb = cp.tile([128, NGRP * 768], FP8)
            table_sb = cp.tile([128, GG * 768], FP8)

            # ---------------- Phase A: conv + gcn linear -> slab ------------
            # t-pair packing: partitions 0:64 hold t, 64:128 hold t+1, so
            # activations run full width and the gcn matmul uses the
            # block-diagonal gwbd moving operand.
            with (
                tc.tile_pool(name="pa", bufs=3) as pa,
                tc.tile_pool(name="psA", bufs=2, space="PSUM") as psA,
                tc.tile_pool(name="psH", bufs=2, space="PSUM") as psH,
            ):
                goff = 0
                for ci, (g0, ng) in enumerate(CHUNKS):
                    w = ng * 128
                    n0 = g0 * 128
                    for tp in range(T // 2):
                        t0 = 2 * tp
                        c0 = t0 * NLP + n0
                        c1 = (t0 + 1) * NLP + n0
                        pf = psA.tile([128, w], F32, tag="pf")
                        nc.tensor.matmul(pf[0:C, :], wf_sb[:], xs_sb[:, c0 : c0 + w],
                                         start=True, stop=True)
                        nc.tensor.matmul(pf[C:, :], wf_sb[:], xs_sb[:, c1 : c1 + w],
                                         start=True, stop=True)
                        pg = psA.tile([128, w], F32, tag="pg")
                        nc.tensor.matmul(pg[0:C, :], wg_sb[:], xs_sb[:, c0 : c0 + w],
                                         start=True, stop=True)
                        nc.tensor.matmul(pg[C:, :], wg_sb[:], xs_sb[:, c1 : c1 + w],
                                         start=True, stop=True)
                        # sigma(z)=0.5(1+tanh(z/2)) with the 0.5s folded into
                        # host weights: h = tf + tf*tg, contracted in two
                        # accumulating matmuls against block-diag W.
                        tf2 = pa.tile([128, w], BF16, tag="tf")
                        nc.scalar.activation(tf2[:], pf[:], AFT.Tanh, bias=biaf_sb[:])
                        tg2 = psA.tile([128, w], F32, tag="tg")
                        nc.scalar.activation(tg2[:], pg[:], AFT.Tanh, bias=biag_sb[:])
                        pr2 = pa.tile([128, w], BF16, tag="pr")
                        nc.vector.tensor_mul(pr2[:], tf2[:], tg2[:])
                        phw = psH.tile([128, w], F32, tag="phw")
                        for j in range(ng):
                            js = slice(j * 128, (j + 1) * 128)
                            nc.tensor.matmul(phw[:, js], tf2[:, js], gwbd_sb[:],
                                             start=True, stop=False)
                            nc.tensor.matmul(phw[:, js], pr2[:, js], gwbd_sb[:],
                                             start=False, stop=True)
                        dstv = slab_sb[:].rearrange("p (g d) -> p g d", d=768)[
                            :, g0 : g0 + ng, t0 * C : (t0 + 2) * C
                        ]
                        nc.vector.tensor_copy(
                            dstv, phw[:].rearrange("p (j z) -> p j z", j=ng)
                        )
                    # chunk complete: slab panel -> DRAM -> AllGather -> SBUF
                    nc.sync.dma_start(
                        slab_cs[ci][:], slab_sb[:, g0 * 768 : (g0 + ng) * 768]
                    )
                    nc.gpsimd.collective_compute(
                        "AllGather",
                        mybir.AluOpType.bypass,
                        replica_groups=[list(range(NCORES))],
                        ins=[slab_cs[ci].opt()],
                        outs=[hw_cs[ci].opt()],
                    )
                    for k in range(NCORES):
                        nc.sync.dma_start(
                            table_sb[:, (goff + k * ng) * 768 : (goff + (k + 1) * ng) * 768],
                            hw_cs[ci][k * 128 : (k + 1) * 128, :],
                        )
                    goff += NCORES * ng

            # ---------------- Phase C: dense adjacency-block matmul ---------
            # agg[dst] = sum_p M[2p+i]^T @ hw[2p+i] per 128-dst window; the
            # fp8 table is SBUF-resident, MD streams pass-major.
            with (
                tc.tile_pool(name="mp", bufs=6) as mp,
                tc.tile_pool(name="ps_c", bufs=1, space="PSUM") as ps_c,
            ):
                tbl = table_sb[:].rearrange("p (g d) -> p g d", d=768)
                xr3 = xr_sb[:].rearrange("p (g d) -> p g d", d=768)
                mdoff = 0
                w0 = 0
                for nw in PASS_WINS:
                    paggs = [
                        (ps_c.tile([128, 512], F32, tag=f"pa{wi}", name=f"pagg_a{wi}"),
                         ps_c.tile([128, 512], F32, tag=f"pb{wi}", name=f"pagg_b{wi}"))
                        for wi in range(nw)
                    ]
                    chb = PCH * 2 * nw * 128
                    for pc in range(NPAIR // PCH):
                        mt = mp.tile([128, PCH, 2, nw * 128], FP8, tag="mt")
                        nc.sync.dma_start(
                            mt[:],
                            MD[:, mdoff : mdoff + chb].rearrange(
                                "p (q i d) -> p q i d", q=PCH, i=2),
                        )
                        mdoff += chb
                        for q in range(PCH):
                            p_ = pc * PCH + q
                            rhs = tbl[:, 2 * p_ : 2 * p_ + 2, :]
                            ss = dict(start=(p_ == 0), stop=(p_ == NPAIR - 1),
                                      perf_mode=DR)
                            for wi in range(nw):
                                lhsT = mt[:, q, :, wi * 128 : (wi + 1) * 128]
                                nc.tensor.matmul(
                                    paggs[wi][0][:, 0:384], lhsT,
                                    rhs[:, :, 0:384], **ss)
                                nc.tensor.matmul(
                                    paggs[wi][1][:, 0:384], lhsT,
                                    rhs[:, :, 384:768], **ss)
                    for wi in range(nw):
                        w_ = w0 + wi
                        fin = fo.tile([128, ROW], F32, tag="fin")
                        # out[n, c*12+t] = agg[n, t*64+c] + (x[n,c,t] + gcn_b[c])
                        xrv3 = xr3[:, w_, :].rearrange("p (c t) -> p c t", c=C)
                        for half, pag in ((0, paggs[wi][0]), (1, paggs[wi][1])):
                            outv = fin[:].rearrange("p (c t) -> p c t", c=C)[
                                :, :, half * 6 : (half + 1) * 6
                            ]
                            inv = pag[:, 0:384].rearrange("p (t d) -> p d t", t=6)
                            nc.vector.tensor_tensor(
                                outv, inv, xrv3[:, :, half * 6 : (half + 1) * 6],
                                mybir.AluOpType.add)
                        nc.sync.dma_start(OUT[w_ * 128 : (w_ + 1) * 128, :], fin[:])
                    w0 += nw

    nc.compile()
    _dedup_ldweights(nc)
    return nc


def _ap_key(ap):
    try:
        return (ap.tensor_name if hasattr(ap, "tensor_name") else str(ap.memref),
                ap.offset, tuple(map(tuple, ap.ap)))
    except Exception:
        return repr(ap)


def _dedup_ldweights(nc):
    """Drop an InstLdweights that reloads exactly the weights already loaded
    by the immediately preceding PE sequence (our paired a/b matmuls share a
    stationary); migrate its semaphore waits to the next kept instruction."""
    removed = 0
    for f in nc.m.functions:
        for blk in f.blocks:
            insts = list(blk.instructions)
            keep = []
            last_key = None
            pending_waits = []
            for inst in insts:
                nm = type(inst).__name__
                if nm == "InstLdweights":
                    key = _ap_key(inst.ins[0])
                    si = inst.sync_info
                    clean = si is None or (not list(si.on_update)
                                           and not list(si.on_wait))
                    # only dedup the fp8 phase-C a/b pairs (baseline-validated
                    # pattern); conv/gcn bf16 reloads are cheap and the
                    # wf/wg-pair dedup is unproven on hardware.
                    try:
                        is_fp8 = inst.ins[0].dtype == mybir.dt.float8e4
                    except Exception:
                        is_fp8 = False
                    if key == last_key and clean and is_fp8:
                        removed += 1
                        continue
                    last_key = key
                elif nm == "InstMatmult":
                    pass  # leaves the loaded weights intact
                elif nm in ("InstEventSemaphore", "InstRegisterMove"):
                    pass  # no PE-array effect
                else:
                    last_key = None
                if pending_waits:
                    si = inst.sync_info
                    if si is None:
                        inst.sync_info = mybir.SyncInfo(
                            on_wait=pending_waits, on_update=[])
                    else:
                        inst.sync_info = mybir.SyncInfo(
                            on_wait=list(si.on_wait) + pending_waits,
                            on_update=list(si.on_update))
                    pending_waits = []
                keep.append(inst)
            if len(keep) != len(insts):
                del blk.instructions[:]
                for i in keep:
                    blk.instructions.append(i)
    return removed


def _table_perm():
    """Global table slot -> padded-global node id, chunk-major:
    gg enumerates (chunk, core, group-within-chunk)."""
    perm = np.empty(NG, np.int64)
    gg = 0
    for g0, ng in CHUNKS:
        for k in range(NCORES):
            for gl in range(ng):
                base = k * NLP + (g0 + gl) * 128
                perm[gg * 128 : (gg + 1) * 128] = base + np.arange(128)
                gg += 1
    return perm


def _prep_inputs(x, filter_w, filter_b, gate_w, gate_b, gcn_w, gcn_b, edge_index):
    x = np.ascontiguousarray(x, dtype=np.float32)
    src = np.asarray(edge_index[0], dtype=np.int64)
    dst = np.asarray(edge_index[1], dtype=np.int64)

    deg = (np.bincount(dst, minlength=N) + 1.0).astype(np.float32)
    dinv = (1.0 / np.sqrt(deg)).astype(np.float32)
    norm_e = dinv[src] * dinv[dst]          # [E]
    self_norm = (1.0 / deg).astype(np.float32)

    srcg_all = (src // NL) * NLP + (src % NL)   # padded global src id

    # conv weights: stacked [current; shifted], sigmoid folded to tanh:
    #   sigma(z) = 0.5 (1 + tanh(z/2)) -> gate weights/bias scaled by 0.5,
    #   the outer 0.5 folded into gcn_w.
    wf = np.concatenate([filter_w[:, :, 1].T, filter_w[:, :, 0].T]).astype(NP_BF16)
    wg = (0.5 * np.concatenate([gate_w[:, :, 1].T, gate_w[:, :, 0].T])).astype(NP_BF16)
    biaf = np.concatenate([filter_b, filter_b]).astype(np.float32).reshape(128, 1)
    biag = (0.5 * np.concatenate([gate_b, gate_b])).astype(np.float32).reshape(128, 1)
    gw_half = 0.5 * np.ascontiguousarray(gcn_w).astype(np.float32)
    gwbd = np.zeros((128, 128), np.float32)
    gwbd[:C, :C] = gw_half
    gwbd[C:, C:] = gw_half
    gwbd = gwbd.astype(NP_BF16)
    bias_row = np.repeat(np.asarray(gcn_b, np.float32), T)  # [768] at (c,t)

    perm = _table_perm()

    in_maps = []
    for k in range(NCORES):
        lo, hi = k * NL, (k + 1) * NL
        xs_n = x[lo:hi]                                 # [1250, 64, 12]
        # xs: [128, COLS] bf16, rows 0:64 current x, 64:128 shifted x,
        # cols (t-major, n-minor): col = t*1280 + n
        xs = np.zeros((128, COLS), np.float32)
        xt = xs_n.transpose(1, 2, 0)                    # [C, T, 1250]
        xs[:C].reshape(C, T, NLP)[:, :, :NL] = xt
        xs[C:].reshape(C, T, NLP)[:, 1:, :NL] = xt[:, :-1, :]
        xr = np.zeros((NLP, ROW), np.float32)
        xr[:NL] = xs_n.reshape(NL, ROW) + bias_row[None, :]
        xr_pm = (xr.reshape(NGRP, 128, ROW).transpose(1, 0, 2)
                 .reshape(128, NGRP * ROW))

        # per-core dense normalized adjacency (dst on this core), self loops
        m = (dst >= lo) & (dst < hi)
        e_src = srcg_all[m]
        e_dstl = (dst[m] - lo).astype(np.int64)
        e_norm = norm_e[m]
        n_ids = np.arange(NL, dtype=np.int64)
        e_src = np.concatenate([e_src, k * NLP + n_ids])
        e_dstl = np.concatenate([e_dstl, n_ids])
        e_norm = np.concatenate([e_norm, self_norm[lo:hi]])
        densem = np.zeros((NG, NLP), np.float32)
        np.add.at(densem, (e_src, e_dstl), e_norm)
        dp = densem[perm]                               # table-slot order

        # MD: [128 slot-partitions, (pass | pair, i, dstcol)] pass-major
        mdcols = []
        w0 = 0
        for nw in PASS_WINS:
            blk = dp.reshape(NPAIR, 2, 128, NLP)[:, :, :, w0 * 128 : (w0 + nw) * 128]
            mdcols.append(blk.transpose(2, 0, 1, 3).reshape(128, NPAIR * 2 * nw * 128))
            w0 += nw
        md = np.concatenate(mdcols, axis=1).astype(NP_FP8)

        in_maps.append({
            "xs": xs.astype(NP_BF16), "wf": wf, "wg": wg,
            "biaf": biaf, "biag": biag, "gwbd": gwbd,
            "md": md, "xr": xr_pm.astype(NP_BF16),
        })
    return in_maps


def benchmark(x, filter_w, filter_b, gate_w, gate_b, gcn_w, gcn_b, edge_index,
              n_lo=8, n_hi=24):
    """Steady-state per-iteration wall time (ns) with device-resident inputs."""
    import time
    import jax
    from jax.experimental.shard_map import shard_map
    from jax.sharding import Mesh, PartitionSpec, NamedSharding
    from concourse import bass2jax as b2j
    import concourse.mybir as mb

    in_maps = _prep_inputs(
        x, filter_w, filter_b, gate_w, gate_b, gcn_w, gcn_b, edge_index
    )
    if "p" not in _prog_cache:
        _prog_cache["p"] = _build_program()
    nc = _prog_cache["p"]
    b2j.install_neuronx_cc_hook()

    in_names, out_names, out_avals, zero_outs = [], [], [], []
    partition_name = nc.partition_id_tensor.name if nc.partition_id_tensor else None
    for alloc in nc.m.functions[0].allocations:
        if not isinstance(alloc, mb.MemoryLocationSet):
            continue
        name = alloc.memorylocations[0].name
        if alloc.kind == "ExternalInput":
            if name != partition_name:
                in_names.append(name)
        elif alloc.kind == "ExternalOutput":
            out_names.append(name)
            shape = tuple(alloc.tensor_shape)
            dtype = mb.dt.np(alloc.dtype)
            out_avals.append(jax.core.ShapedArray(shape, dtype))
            zero_outs.append(np.zeros(shape, dtype))
    n_params = len(in_names)
    all_names = in_names + out_names
    if partition_name is not None:
        all_names.append(partition_name)

    def _body(*args):
        operands = list(args)
        if partition_name is not None:
            operands.append(b2j.partition_id_tensor())
        return tuple(b2j._bass_exec_p.bind(
            *operands,
            out_avals=tuple(out_avals),
            in_names=tuple(all_names),
            out_names=tuple(out_names),
            lowering_input_output_aliases=(),
            sim_require_finite=True,
            sim_require_nnan=True,
            nc=nc,
        ))

    devices = jax.devices()[:NCORES]
    mesh = Mesh(np.asarray(devices), ("core",))
    nin = n_params + len(zero_outs)
    sharded = jax.jit(
        shard_map(_body, mesh=mesh,
                  in_specs=(PartitionSpec("core"),) * nin,
                  out_specs=(PartitionSpec("core"),) * len(out_names),
                  check_rep=False),
        keep_unused=True,
    )
    sh = NamedSharding(mesh, PartitionSpec("core"))
    args = [
        jax.device_put(
            np.concatenate([np.asarray(in_maps[c][n]) for c in range(NCORES)], 0), sh)
        for n in in_names
    ] + [
        jax.device_put(np.zeros((NCORES * z.shape[0], *z.shape[1:]), z.dtype), sh)
        for z in zero_outs
    ]

    def run(n):
        t0 = time.perf_counter()
        outs = None
        for _ in range(n):
            outs = sharded(*args)
        jax.block_until_ready(outs)
        return (time.perf_counter() - t0) * 1e9

    run(6)  # warmup
    ests = []
    for _ in range(2):
        t_lo = run(40)
        t_hi = run(120)
        ests.append((t_hi - t_lo) / 80)
    return min(ests), max(ests)


def _install_ntff_shim():
    """bass_utils wants antenv.axon_hooks (absent in this image); rebuild the
    NTFF profile hook via ctypes against libaxon_pjrt.so and inject it."""
    import sys
    import types

    if "antenv.axon_hooks" in sys.modules:
        return
    try:
        sys.path.insert(0, "/root/.axon_site")
        from trn_agent_boot.trn_boot import _ntff_profile_via_ctypes

        hook = _ntff_profile_via_ctypes("/opt/axon/libaxon_pjrt.so")
        mod = types.ModuleType("antenv.axon_hooks")
        mod.get_axon_ntff_profile_hook = lambda: hook
        mod.set_axon_ntff_profile_hook = lambda h: None
        import antenv  # noqa: F401  (ensure parent package importable)

        sys.modules["antenv.axon_hooks"] = mod
    except Exception as e:  # pragma: no cover - profiling is best-effort
        print(f"ntff shim failed: {e}", file=sys.stderr)


def kernel(x, filter_w, filter_b, gate_w, gate_b, gcn_w, gcn_b, edge_index):
    global LAST_EXEC_NS, LAST_RESULTS
    in_maps = _prep_inputs(
        x, filter_w, filter_b, gate_w, gate_b, gcn_w, gcn_b, edge_index
    )
    if "p" not in _prog_cache:
        _prog_cache["p"] = _build_program()
    nc = _prog_cache["p"]

    trace = os.environ.get("KBENCH_TRACE", "0") == "1"
    if trace:
        _install_ntff_shim()
    res = run_bass_kernel_spmd(
        nc, in_maps, core_ids=list(range(NCORES)), trace=trace,
        trace_cores=list(range(NCORES)) if trace else None,
    )
    LAST_EXEC_NS = res.exec_time_ns
    LAST_RESULTS = res
    out = np.empty((N, C, T), np.float32)
    for k in range(NCORES):
        rows = res.results[k]["out"][:NL]         # [1250, 768] (c-major, t-minor)
        out[k * NL : (k + 1) * NL] = rows.reshape(NL, C, T)
    return out


# revision 27
# speedup vs baseline: 1.0580x; 1.0580x over previous
"""GraphWaveNet layer on 8 Trainium2 NeuronCores.

Sharding: nodes partitioned across 8 cores (1250 each, padded to 1280).
v2 design (vs v0 baseline at ~432us):
  Phase A (node-chunk-major, t-pair packed): gated causal conv with both
      tanh activations running full 128-partition width by packing t and
      t+1 into the two partition halves; gcn linear via a block-diagonal
      [[W,0],[0,W]] moving operand so the (t,t+1) pair needs one
      stationary load per data chunk.  Produces the fp8 hw slab
      [128, 10 groups x 768] partition-major in SBUF.
  AllGather (chunked, 3 chunks of 4/4/2 node groups): each chunk's slab
      panel is DMA'd to DRAM and all-gathered as soon as phase A finishes
      that node chunk, overlapping the collective with the rest of
      phase A and with phase C's start.  Gathered panels are loaded into
      a persistent SBUF table [128, 80 groups x 768] (61.4KB/partition)
      with one contiguous descriptor per (chunk, core).
  Phase C (dense adjacency matmul, table resident in SBUF): 3 passes of
      4/4/2 dst windows (PSUM: 2 banks per window).  The normalized dense
      adjacency MD streams from DRAM pass-major with partition-contiguous
      4KB descriptors and deep prefetch, so the PE runs back-to-back fp8
      DoubleRow matmuls (no HBM re-streaming of the table).  Fused
      permute + residual + bias epilogue per window.
"""

import os
import numpy as np
import ml_dtypes

import concourse.bass as bass
import concourse.bacc as bacc
import concourse.mybir as mybir
import concourse.tile as tile
from concourse.bass_utils import run_bass_kernel_spmd

N, C, T, E = 10000, 64, 12, 160000
NCORES = 8
NL = N // NCORES            # 1250 real nodes per core
NLP = 1280                  # padded nodes per core
NG = NCORES * NLP           # 10240 padded global nodes
ROW = C * T                 # 768 elems per hw row, (t-major, c-minor)
COLS = NLP * T              # 15360 cols, (t-major, n-minor): col = t*1280 + n
WINS = NLP // 128           # 10 dst windows per core
NGRP = NLP // 128           # 10 node groups per core
NPAIR = NG // 256           # 40 src row-pair blocks
GG = NG // 128              # 80 global table groups

CHUNKS = ((0, 4), (4, 4), (8, 2))   # (first group, ngroups) per AG chunk
PASS_WINS = (4, 4, 2)               # dst windows per PSUM pass
PCH = 4                             # pairs per MD stream chunk

F32 = mybir.dt.float32
BF16 = mybir.dt.bfloat16
FP8 = mybir.dt.float8e4
NP_BF16 = ml_dtypes.bfloat16
NP_FP8 = ml_dtypes.float8_e4m3

MD_BYTES = sum(NPAIR * 2 * nw * 128 for nw in PASS_WINS)  # per partition

LAST_EXEC_NS = None
LAST_RESULTS = None

_prog_cache = {}


def _build_program():
    AFT = mybir.ActivationFunctionType
    DR = mybir.MatmulPerfMode.DoubleRow

    nc = bacc.Bacc(None, num_devices=NCORES, dynamic_dma_scratch_size=16384)
    XS = nc.dram_tensor("xs", [128, COLS], BF16, kind="ExternalInput")
    WF = nc.dram_tensor("wf", [2 * C, C], BF16, kind="ExternalInput")
    WG = nc.dram_tensor("wg", [2 * C, C], BF16, kind="ExternalInput")
    BIAF = nc.dram_tensor("biaf", [128, 1], F32, kind="ExternalInput")
    BIAG = nc.dram_tensor("biag", [128, 1], F32, kind="ExternalInput")
    GWBD = nc.dram_tensor("gwbd", [128, 128], BF16, kind="ExternalInput")
    MD = nc.dram_tensor("md", [128, MD_BYTES], FP8, kind="ExternalInput")
    XR = nc.dram_tensor("xr", [128, NGRP * ROW], BF16, kind="ExternalInput")
    OUT = nc.dram_tensor("out", [NLP, ROW], F32, kind="ExternalOutput")

    with tile.TileContext(nc) as tc:
        with (
            tc.tile_pool(name="dram", bufs=1, space="DRAM") as dram,
            tc.tile_pool(name="const", bufs=1) as cp,
            tc.tile_pool(name="fo", bufs=3) as fo,
        ):
            slab_cs = [dram.tile([128, ng * 768], FP8, name=f"slab_c{i}")
                       for i, (g0, ng) in enumerate(CHUNKS)]
            hw_cs = [dram.tile([NCORES * 128, ng * 768], FP8,
                               addr_space="Shared", name=f"hw_c{i}")
                     for i, (g0, ng) in enumerate(CHUNKS)]

            # xs gates phase A: load per node-chunk (strided over t) so the
            # first chunk's convs start after ~1/3 of the transfer.
            xs_sb = cp.tile([128, COLS], BF16)
            xs3 = xs_sb[:].rearrange("p (t n) -> p t n", n=NLP)
            XS3 = XS[:].rearrange("p (t n) -> p t n", n=NLP)
            for g0, ng in CHUNKS:
                nc.sync.dma_start(
                    xs3[:, :, g0 * 128 : (g0 + ng) * 128],
                    XS3[:, :, g0 * 128 : (g0 + ng) * 128],
                )

            wf_sb = cp.tile([2 * C, C], BF16)
            wg_sb = cp.tile([2 * C, C], BF16)
            biaf_sb = cp.tile([128, 1], F32)
            biag_sb = cp.tile([128, 1], F32)
            gwbd_sb = cp.tile([128, 128], BF16)
            xr_sb = cp.tile([128, NGRP * ROW], BF16)
            for t_, d_ in ((wf_sb, WF), (wg_sb, WG), (biaf_sb, BIAF),
                           (biag_sb, BIAG), (gwbd_sb, GWBD), (xr_sb, XR)):
                nc.sync.dma_start(t_[:], d_[:])

            slab_sb = cp.tile([128, NGRP * 768], FP8)
            table_sb = cp.tile([128, GG * 768], FP8)

            # ---------------- Phase A: conv + gcn linear -> slab ------------
            # t-pair packing: partitions 0:64 hold t, 64:128 hold t+1, so
            # activations run full width and the gcn matmul uses the
            # block-diagonal gwbd moving operand.
            with (
                tc.tile_pool(name="pa", bufs=3) as pa,
                tc.tile_pool(name="psA", bufs=2, space="PSUM") as psA,
                tc.tile_pool(name="psH", bufs=2, space="PSUM") as psH,
            ):
                goff = 0
                for ci, (g0, ng) in enumerate(CHUNKS):
                    w = ng * 128
                    n0 = g0 * 128
                    for tp in range(T // 2):
                        t0 = 2 * tp
                        c0 = t0 * NLP + n0
                        c1 = (t0 + 1) * NLP + n0
                        pf = psA.tile([128, w], F32, tag="pf")
                        nc.tensor.matmul(pf[0:C, :], wf_sb[:], xs_sb[:, c0 : c0 + w],
                                         start=True, stop=True)
                        nc.tensor.matmul(pf[C:, :], wf_sb[:], xs_sb[:, c1 : c1 + w],
                                         start=True, stop=True)
                        pg = psA.tile([128, w], F32, tag="pg")
                        nc.tensor.matmul(pg[0:C, :], wg_sb[:], xs_sb[:, c0 : c0 + w],
                                         start=True, stop=True)
                        nc.tensor.matmul(pg[C:, :], wg_sb[:], xs_sb[:, c1 : c1 + w],
                                         start=True, stop=True)
                        # sigma(z)=0.5(1+tanh(z/2)) with the 0.5s folded into
                        # host weights: h = tf + tf*tg, contracted in two
                        # accumulating matmuls against block-diag W.
                        tf2 = pa.tile([128, w], BF16, tag="tf")
                        nc.scalar.activation(tf2[:], pf[:], AFT.Tanh, bias=biaf_sb[:])
                        tg2 = psA.tile([128, w], F32, tag="tg")
                        nc.scalar.activation(tg2[:], pg[:], AFT.Tanh, bias=biag_sb[:])
                        pr2 = pa.tile([128, w], BF16, tag="pr")
                        nc.vector.tensor_mul(pr2[:], tf2[:], tg2[:])
                        phw = psH.tile([128, w], F32, tag="phw")
                        for j in range(ng):
                            js = slice(j * 128, (j + 1) * 128)
                            nc.tensor.matmul(phw[:, js], tf2[:, js], gwbd_sb[:],
                                             start=True, stop=False)
                            nc.tensor.matmul(phw[:, js], pr2[:, js], gwbd_sb[:],
                                             start=False, stop=True)
                        dstv = slab_sb[:].rearrange("p (g d) -> p g d", d=768)[
                            :, g0 : g0 + ng, t0 * C : (t0 + 2) * C
                        ]
                        nc.vector.tensor_copy(
                            dstv, phw[:].rearrange("p (j z) -> p j z", j=ng)
                        )
                    # chunk complete: slab panel -> DRAM -> AllGather -> SBUF
                    nc.sync.dma_start(
                        slab_cs[ci][:], slab_sb[:, g0 * 768 : (g0 + ng) * 768]
                    )
                    nc.gpsimd.collective_compute(
                        "AllGather",
                        mybir.AluOpType.bypass,
                        replica_groups=[list(range(NCORES))],
                        ins=[slab_cs[ci].opt()],
                        outs=[hw_cs[ci].opt()],
                    )
                    for k in range(NCORES):
                        nc.sync.dma_start(
                            table_sb[:, (goff + k * ng) * 768 : (goff + (k + 1) * ng) * 768],
                            hw_cs[ci][k * 128 : (k + 1) * 128, :],
                        )
                    goff += NCORES * ng

            # ---------------- Phase C: dense adjacency-block matmul ---------
            # agg[dst] = sum_p M[2p+i]^T @ hw[2p+i] per 128-dst window; the
            # fp8 table is SBUF-resident, MD streams pass-major.
            with (
                tc.tile_pool(name="mp", bufs=6) as mp,
                tc.tile_pool(name="ps_c", bufs=1, space="PSUM") as ps_c,
            ):
                tbl = table_sb[:].rearrange("p (g d) -> p g d", d=768)
                xr3 = xr_sb[:].rearrange("p (g d) -> p g d", d=768)
                mdoff = 0
                w0 = 0
                for nw in PASS_WINS:
                    paggs = [
                        (ps_c.tile([128, 512], F32, tag=f"pa{wi}", name=f"pagg_a{wi}"),
                         ps_c.tile([128, 512], F32, tag=f"pb{wi}", name=f"pagg_b{wi}"))
                        for wi in range(nw)
                    ]
                    chb = PCH * 2 * nw * 128
                    for pc in range(NPAIR // PCH):
                        mt = mp.tile([128, PCH, 2, nw * 128], FP8, tag="mt")
                        nc.sync.dma_start(
                            mt[:],
                            MD[:, mdoff : mdoff + chb].rearrange(
                                "p (q i d) -> p q i d", q=PCH, i=2),
                        )
                        mdoff += chb
                        for q in range(PCH):
                            p_ = pc * PCH + q
                            rhs = tbl[:, 2 * p_ : 2 * p_ + 2, :]
                            ss = dict(start=(p_ == 0), stop=(p_ == NPAIR - 1),
                                      perf_mode=DR)
                            for wi in range(nw):
                                lhsT = mt[:, q, :, wi * 128 : (wi + 1) * 128]
                                nc.tensor.matmul(
                                    paggs[wi][0][:, 0:384], lhsT,
                                    rhs[:, :, 0:384], **ss)
                                nc.tensor.matmul(
                                    paggs[wi][1][:, 0:384], lhsT,
                                    rhs[:, :, 384:768], **ss)
                    for wi in range(nw):
                        w_ = w0 + wi
                        fin = fo.tile([128, ROW], F32, tag="fin")
                        # out[n, c*12+t] = agg[n, t*64+c] + (x[n,c,t] + gcn_b[c])
                        xrv3 = xr3[:, w_, :].rearrange("p (c t) -> p c t", c=C)
                        for half, pag in ((0, paggs[wi][0]), (1, paggs[wi][1])):
                            outv = fin[:].rearrange("p (c t) -> p c t", c=C)[
                                :, :, half * 6 : (half + 1) * 6
                            ]
                            inv = pag[:, 0:384].rearrange("p (t d) -> p d t", t=6)
                            nc.vector.tensor_tensor(
                                outv, inv, xrv3[:, :, half * 6 : (half + 1) * 6],
                                mybir.AluOpType.add)
                        nc.sync.dma_start(OUT[w_ * 128 : (w_ + 1) * 128, :], fin[:])
                    w0 += nw

    nc.compile()
    _dedup_ldweights(nc)
    return nc


def _ap_key(ap):
    try:
        return (ap.tensor_name if hasattr(ap, "tensor_name") else str(ap.memref),
                ap.offset, tuple(map(tuple, ap.ap)))
    except Exception:
        return repr(ap)


def _dedup_ldweights(nc):
    """Drop an InstLdweights that reloads exactly the weights already loaded
    by the immediately preceding PE sequence (our paired a/b matmuls share a
    stationary); migrate its semaphore waits to the next kept instruction."""
    removed = 0
    for f in nc.m.functions:
        for blk in f.blocks:
            insts = list(blk.instructions)
            keep = []
            last_key = None
            pending_waits = []
            for inst in insts:
                nm = type(inst).__name__
                if nm == "InstLdweights":
                    key = _ap_key(inst.ins[0])
                    si = inst.sync_info
                    clean = si is None or (not list(si.on_update)
                                           and not list(si.on_wait))
                    # only dedup the fp8 phase-C a/b pairs (baseline-validated
                    # pattern); conv/gcn bf16 reloads are cheap and the
                    # wf/wg-pair dedup is unproven on hardware.
                    try:
                        is_fp8 = inst.ins[0].dtype == mybir.dt.float8e4
                    except Exception:
                        is_fp8 = False
                    if key == last_key and clean and is_fp8:
                        removed += 1
                        continue
                    last_key = key
                elif nm == "InstMatmult":
                    pass  # leaves the loaded weights intact
                elif nm in ("InstEventSemaphore", "InstRegisterMove"):
                    pass  # no PE-array effect
                else:
                    last_key = None
                if pending_waits:
                    si = inst.sync_info
                    if si is None:
                        inst.sync_info = mybir.SyncInfo(
                            on_wait=pending_waits, on_update=[])
                    else:
                        inst.sync_info = mybir.SyncInfo(
                            on_wait=list(si.on_wait) + pending_waits,
                            on_update=list(si.on_update))
                    pending_waits = []
                keep.append(inst)
            if len(keep) != len(insts):
                del blk.instructions[:]
                for i in keep:
                    blk.instructions.append(i)
    return removed


def _table_perm():
    """Global table slot -> padded-global node id, chunk-major:
    gg enumerates (chunk, core, group-within-chunk)."""
    perm = np.empty(NG, np.int64)
    gg = 0
    for g0, ng in CHUNKS:
        for k in range(NCORES):
            for gl in range(ng):
                base = k * NLP + (g0 + gl) * 128
                perm[gg * 128 : (gg + 1) * 128] = base + np.arange(128)
                gg += 1
    return perm


def _prep_inputs(x, filter_w, filter_b, gate_w, gate_b, gcn_w, gcn_b, edge_index):
    x = np.ascontiguousarray(x, dtype=np.float32)
    src = np.asarray(edge_index[0], dtype=np.int64)
    dst = np.asarray(edge_index[1], dtype=np.int64)

    deg = (np.bincount(dst, minlength=N) + 1.0).astype(np.float32)
    dinv = (1.0 / np.sqrt(deg)).astype(np.float32)
    norm_e = dinv[src] * dinv[dst]          # [E]
    self_norm = (1.0 / deg).astype(np.float32)

    srcg_all = (src // NL) * NLP + (src % NL)   # padded global src id

    # conv weights: stacked [current; shifted], sigmoid folded to tanh:
    #   sigma(z) = 0.5 (1 + tanh(z/2)) -> gate weights/bias scaled by 0.5,
    #   the outer 0.5 folded into gcn_w.
    wf = np.concatenate([filter_w[:, :, 1].T, filter_w[:, :, 0].T]).astype(NP_BF16)
    wg = (0.5 * np.concatenate([gate_w[:, :, 1].T, gate_w[:, :, 0].T])).astype(NP_BF16)
    biaf = np.concatenate([filter_b, filter_b]).astype(np.float32).reshape(128, 1)
    biag = (0.5 * np.concatenate([gate_b, gate_b])).astype(np.float32).reshape(128, 1)
    gw_half = 0.5 * np.ascontiguousarray(gcn_w).astype(np.float32)
    gwbd = np.zeros((128, 128), np.float32)
    gwbd[:C, :C] = gw_half
    gwbd[C:, C:] = gw_half
    gwbd = gwbd.astype(NP_BF16)
    bias_row = np.repeat(np.asarray(gcn_b, np.float32), T)  # [768] at (c,t)

    perm = _table_perm()

    in_maps = []
    for k in range(NCORES):
        lo, hi = k * NL, (k + 1) * NL
        xs_n = x[lo:hi]                                 # [1250, 64, 12]
        # xs: [128, COLS] bf16, rows 0:64 current x, 64:128 shifted x,
        # cols (t-major, n-minor): col = t*1280 + n
        xs = np.zeros((128, COLS), np.float32)
        xt = xs_n.transpose(1, 2, 0)                    # [C, T, 1250]
        xs[:C].reshape(C, T, NLP)[:, :, :NL] = xt
        xs[C:].reshape(C, T, NLP)[:, 1:, :NL] = xt[:, :-1, :]
        xr = np.zeros((NLP, ROW), np.float32)
        xr[:NL] = xs_n.reshape(NL, ROW) + bias_row[None, :]
        xr_pm = (xr.reshape(NGRP, 128, ROW).transpose(1, 0, 2)
                 .reshape(128, NGRP * ROW))

        # per-core dense normalized adjacency (dst on this core), self loops
        m = (dst >= lo) & (dst < hi)
        e_src = srcg_all[m]
        e_dstl = (dst[m] - lo).astype(np.int64)
        e_norm = norm_e[m]
        n_ids = np.arange(NL, dtype=np.int64)
        e_src = np.concatenate([e_src, k * NLP + n_ids])
        e_dstl = np.concatenate([e_dstl, n_ids])
        e_norm = np.concatenate([e_norm, self_norm[lo:hi]])
        densem = np.zeros((NG, NLP), np.float32)
        np.add.at(densem, (e_src, e_dstl), e_norm)
        dp = densem[perm]                               # table-slot order

        # MD: [128 slot-partitions, (pass | pair, i, dstcol)] pass-major
        mdcols = []
        w0 = 0
        for nw in PASS_WINS:
            blk = dp.reshape(NPAIR, 2, 128, NLP)[:, :, :, w0 * 128 : (w0 + nw) * 128]
            mdcols.append(blk.transpose(2, 0, 1, 3).reshape(128, NPAIR * 2 * nw * 128))
            w0 += nw
        md = np.concatenate(mdcols, axis=1).astype(NP_FP8)

        in_maps.append({
            "xs": xs.astype(NP_BF16), "wf": wf, "wg": wg,
            "biaf": biaf, "biag": biag, "gwbd": gwbd,
            "md": md, "xr": xr_pm.astype(NP_BF16),
        })
    return in_maps


def benchmark(x, filter_w, filter_b, gate_w, gate_b, gcn_w, gcn_b, edge_index,
              n_lo=8, n_hi=24):
    """Steady-state per-iteration wall time (ns) with device-resident inputs."""
    import time
    import jax
    from jax.experimental.shard_map import shard_map
    from jax.sharding import Mesh, PartitionSpec, NamedSharding
    from concourse import bass2jax as b2j
    import concourse.mybir as mb

    in_maps = _prep_inputs(
        x, filter_w, filter_b, gate_w, gate_b, gcn_w, gcn_b, edge_index
    )
    if "p" not in _prog_cache:
        _prog_cache["p"] = _build_program()
    nc = _prog_cache["p"]
    b2j.install_neuronx_cc_hook()

    in_names, out_names, out_avals, zero_outs = [], [], [], []
    partition_name = nc.partition_id_tensor.name if nc.partition_id_tensor else None
    for alloc in nc.m.functions[0].allocations:
        if not isinstance(alloc, mb.MemoryLocationSet):
            continue
        name = alloc.memorylocations[0].name
        if alloc.kind == "ExternalInput":
            if name != partition_name:
                in_names.append(name)
        elif alloc.kind == "ExternalOutput":
            out_names.append(name)
            shape = tuple(alloc.tensor_shape)
            dtype = mb.dt.np(alloc.dtype)
            out_avals.append(jax.core.ShapedArray(shape, dtype))
            zero_outs.append(np.zeros(shape, dtype))
    n_params = len(in_names)
    all_names = in_names + out_names
    if partition_name is not None:
        all_names.append(partition_name)

    def _body(*args):
        operands = list(args)
        if partition_name is not None:
            operands.append(b2j.partition_id_tensor())
        return tuple(b2j._bass_exec_p.bind(
            *operands,
            out_avals=tuple(out_avals),
            in_names=tuple(all_names),
            out_names=tuple(out_names),
            lowering_input_output_aliases=(),
            sim_require_finite=True,
            sim_require_nnan=True,
            nc=nc,
        ))

    devices = jax.devices()[:NCORES]
    mesh = Mesh(np.asarray(devices), ("core",))
    nin = n_params + len(zero_outs)
    sharded = jax.jit(
        shard_map(_body, mesh=mesh,
                  in_specs=(PartitionSpec("core"),) * nin,
                  out_specs=(PartitionSpec("core"),) * len(out_names),
                  check_rep=False),
        keep_unused=True,
    )
    sh = NamedSharding(mesh, PartitionSpec("core"))
    args = [
        jax.device_put(
            np.concatenate([np.asarray(in_maps[c][n]) for c in range(NCORES)], 0), sh)
        for n in in_names
    ] + [
        jax.device_put(np.zeros((NCORES * z.shape[0], *z.shape[1:]), z.dtype), sh)
        for z in zero_outs
    ]

    def run(n):
        t0 = time.perf_counter()
        outs = None
        for _ in range(n):
            outs = sharded(*args)
        jax.block_until_ready(outs)
        return (time.perf_counter() - t0) * 1e9

    run(6)  # warmup
    ests = []
    for _ in range(2):
        t_lo = run(40)
        t_hi = run(120)
        ests.append((t_hi - t_lo) / 80)
    return min(ests), max(ests)


def _install_ntff_shim():
    """bass_utils wants antenv.axon_hooks (absent in this image); rebuild the
    NTFF profile hook via ctypes against libaxon_pjrt.so and inject it."""
    import sys
    import types

    if "antenv.axon_hooks" in sys.modules:
        return
    try:
        sys.path.insert(0, "/root/.axon_site")
        from trn_agent_boot.trn_boot import _ntff_profile_via_ctypes

        hook = _ntff_profile_via_ctypes("/opt/axon/libaxon_pjrt.so")
        mod = types.ModuleType("antenv.axon_hooks")
        mod.get_axon_ntff_profile_hook = lambda: hook
        mod.set_axon_ntff_profile_hook = lambda h: None
        import antenv  # noqa: F401  (ensure parent package importable)

        sys.modules["antenv.axon_hooks"] = mod
    except Exception as e:  # pragma: no cover - profiling is best-effort
        print(f"ntff shim failed: {e}", file=sys.stderr)


def kernel(x, filter_w, filter_b, gate_w, gate_b, gcn_w, gcn_b, edge_index):
    global LAST_EXEC_NS, LAST_RESULTS
    in_maps = _prep_inputs(
        x, filter_w, filter_b, gate_w, gate_b, gcn_w, gcn_b, edge_index
    )
    if "p" not in _prog_cache:
        _prog_cache["p"] = _build_program()
    nc = _prog_cache["p"]

    trace = os.environ.get("KBENCH_TRACE", "0") == "1"
    if trace:
        _install_ntff_shim()
    res = run_bass_kernel_spmd(
        nc, in_maps, core_ids=list(range(NCORES)), trace=trace,
        trace_cores=list(range(NCORES)) if trace else None,
    )
    LAST_EXEC_NS = res.exec_time_ns
    LAST_RESULTS = res
    out = np.empty((N, C, T), np.float32)
    for k in range(NCORES):
        rows = res.results[k]["out"][:NL]         # [1250, 768] (c-major, t-minor)
        out[k * NL : (k + 1) * NL] = rows.reshape(NL, C, T)
    return out
